# revision 42
# baseline (speedup 1.0000x reference)
"""Distributed single-head attention for Trainium2 (8 NeuronCores).

Problem: B=4, S=2048, D=1024 fp32 attention:
    q = x@Wq+bq; k = x@Wk+bk; v = x@Wv+bv
    out = softmax(q k^T / sqrt(D) + mask) v

Sharding: core c handles batch c//2, query rows [1024*(c%2), +1024).
The in-pair work split (pair = the two cores of one batch):
  - scores/softmax/PV: each core does its own 1024 query rows over ALL
    2048 keys (x for the whole batch is replicated to the pair as fp8,
    plus a bf16 copy of just the core's own columns, `xq`).
  - V projection: each core projects only its OWN 1024 keys' V and the
    halves are exchanged with ONE in-pair AllGather, overlapped under
    the TT projection + first scores phases. A zero-byte-ish dummy
    AllGather is issued at t=0 so the one-time global CC barrier
    (~20us) burns during the V projection instead of on the critical
    path.
Using a separate `xq` input makes the program identical for both pair
ranks with NATURAL key order everywhere (the AllGather's rank order ==
score column order), so one compiled program serves all 8 cores.

Algebraic tricks:
  - K projection eliminated: scores (up to a per-row constant softmax
    drops) = TT @ x^T with host-precomputed M2 = Wq@Wk^T, w2 = Wk@bq,
    TT = x_q @ M2 + w2.
  - V bias folded out: softmax rows sum to 1 => attn@(xWv+bv)/rsum ==
    attn@(xWv)/rsum + bv; the host adds bv to the final output exactly.

Precision (HW == numpy sim to 3 decimals; gate 2e-2):
  - scores matmul fp8e4m3 DoubleRow (1.0e-2 alone)
  - PV: key chunks 10..15 fp8 DoubleRow, 0..9 bf16 -> 1.674e-2 total
  - everything else bf16 (fp8 on the projections blows the gate)

Per-core PE stream (~592 matmuls at ~216ns):
  V own:   vo[k][128,1024] = xq^T Wv, k=0..7      (128 MM)
  TT:      tt8 fp8 = M2^T xq + w2                 (128 MM)
  scores:  fp8 DoubleRow per q-chunk              (128 MM)
  PV:      10 bf16 + 3 fp8-DR pairs per q-chunk   (208 MM)
"""

from contextlib import ExitStack

import numpy as np
import ml_dtypes

import concourse.bass as bass
import concourse.tile as tile
import concourse.mybir as mybir
from concourse import bacc
from concourse.bass_utils import run_bass_kernel_spmd

BF16 = mybir.dt.bfloat16
FP8 = mybir.dt.float8e4
F32 = mybir.dt.float32
AF = mybir.ActivationFunctionType
DR = mybir.MatmulPerfMode.DoubleRow

D = 1024  # model dim (= contraction dim)
S = 2048  # full sequence (keys)
Q = 1024  # queries per core
P = 128  # partitions
ND = D // P  # 8 d-chunks
NS = S // P  # 16 key chunks
NQ = Q // P  # 8 query chunks
NBF = 10  # key chunks kept bf16 in PV; chunks NBF..15 go fp8 DoubleRow
SCALE = 1.0 / float(np.sqrt(np.float32(D)))
PAIRS = [[0, 1], [2, 3], [4, 5], [6, 7]]

_NC_CACHE: dict[bool, bacc.Bacc] = {}


def _build(use_mask: bool) -> bacc.Bacc:
    nc = bacc.Bacc("TRN2", target_bir_lowering=False, debug=False, num_devices=8)

    xq_d = nc.dram_tensor("xq", [D, Q], BF16, kind="ExternalInput")
    x8_d = nc.dram_tensor("x8", [D, S], FP8, kind="ExternalInput")
    m2_d = nc.dram_tensor("m2", [D, D], BF16, kind="ExternalInput")
    wv_d = nc.dram_tensor("wv", [D, D], BF16, kind="ExternalInput")
    w2_d = nc.dram_tensor("w22", [P, ND], F32, kind="ExternalInput")
    if use_mask:
        mask_d = nc.dram_tensor("maskp", [Q, S], F32, kind="ExternalInput")
    out_d = nc.dram_tensor("out", [Q, D], F32, kind="ExternalOutput")

    # collective bounce buffers + a tiny dummy to pre-fire the CC barrier
    dmy_in = nc.dram_tensor("dmy_in", [1, 64], BF16)
    dmy_out = nc.dram_tensor("dmy_out", [2, 64], BF16)
    vb_in = nc.dram_tensor("vb_in", [Q, D], BF16)
    vb_out = nc.dram_tensor("vb_out", [S, D], BF16)

    with tile.TileContext(nc) as tc, ExitStack() as ctx:
        xq_pool = ctx.enter_context(tc.tile_pool(name="xq", bufs=ND))
        x8_pool = ctx.enter_context(tc.tile_pool(name="x8", bufs=ND // 2))
        m2_pool = ctx.enter_context(tc.tile_pool(name="m2", bufs=ND))
        wv_pool = ctx.enter_context(tc.tile_pool(name="wv", bufs=ND))
        tt_pool = ctx.enter_context(tc.tile_pool(name="tt", bufs=ND // 2))
        vo_pool = ctx.enter_context(tc.tile_pool(name="vo", bufs=ND))
        vt_pool = ctx.enter_context(tc.tile_pool(name="vt", bufs=NBF))
        vs_pool = ctx.enter_context(tc.tile_pool(name="vs", bufs=2))
        v8_pool = ctx.enter_context(tc.tile_pool(name="v8", bufs=3))
        at8_pool = ctx.enter_context(tc.tile_pool(name="at8", bufs=2))
        const_pool = ctx.enter_context(tc.tile_pool(name="const", bufs=1))
        exp_pool = ctx.enter_context(tc.tile_pool(name="exp", bufs=4))
        at_pool = ctx.enter_context(tc.tile_pool(name="at", bufs=2))
        stat_pool = ctx.enter_context(tc.tile_pool(name="stat", bufs=12))
        o_pool = ctx.enter_context(tc.tile_pool(name="o", bufs=2))
        if use_mask:
            m_pool = ctx.enter_context(tc.tile_pool(name="m", bufs=2))
        psum = ctx.enter_context(tc.tile_pool(name="psum", bufs=4, space="PSUM"))

        # ---- dummy collective first: absorbs the one-time global CC
        # barrier while the V projection runs ----
        nc.gpsimd.collective_compute(
            "AllGather",
            mybir.AluOpType.bypass,
            replica_groups=PAIRS,
            ins=[dmy_in.ap().opt()],
            outs=[dmy_out.ap().opt()],
        )

        # ---- input loads ----
        # gpsimd: xq halves (V+TT critical path), w22, then x8 (first read
        #         only at the scores phase ~70us in)
        # sync/scalar: wv halves split over both queues, then m2 on sync
        xq = [xq_pool.tile([P, Q], BF16, tag="xq", name=f"xq{i}") for i in range(ND)]
        wv = [wv_pool.tile([P, D], BF16, tag="wv", name=f"wv{i}") for i in range(ND)]
        m2 = [m2_pool.tile([P, D], BF16, tag="m2", name=f"m2{i}") for i in range(ND)]
        for h in range(2):
            for d in range(ND):
                nc.gpsimd.dma_start(
                    xq[d][:, h * 512 : (h + 1) * 512],
                    xq_d[d * P : (d + 1) * P, h * 512 : (h + 1) * 512],
                )
        for d in range(ND):
            q = nc.sync if d % 2 == 0 else nc.scalar
            for n in range(2):
                q.dma_start(
                    wv[d][:, n * 512 : (n + 1) * 512],
                    wv_d[d * P : (d + 1) * P, n * 512 : (n + 1) * 512],
                )
        for d in range(ND):
            nc.sync.dma_start(m2[d][:], m2_d[d * P : (d + 1) * P, :])

        x8 = [
            x8_pool.tile([P, 2 * S], FP8, tag="x8", name=f"x8_{i}")
            for i in range(ND // 2)
        ]
        w2_sb = const_pool.tile([P, ND], F32, tag="w2")
        nc.gpsimd.dma_start(w2_sb[:], w2_d[:, :])
        for e2 in range(ND // 2):
            for j in range(2):
                nc.gpsimd.dma_start(
                    x8[e2][:, j * S : (j + 1) * S],
                    x8_d[(2 * e2 + j) * P : (2 * e2 + j + 1) * P, :],
                )

        # ---- V own-half proj: vo[k] = xq^T Wv, k=0..7, then bounce out ----
        vo = [vo_pool.tile([P, D], BF16, tag="vo", name=f"vo{i}") for i in range(ND)]
        for kb in range(2):
            pss = [
                psum.tile([P, D], F32, tag="ps", name=f"vps{kb}_{j}")
                for j in range(4)
            ]
            for d in range(ND):
                for j in range(4):
                    k = kb * 4 + j
                    for n in range(2):
                        nc.tensor.matmul(
                            pss[j][:, n * 512 : (n + 1) * 512],
                            lhsT=xq[d][:, k * P : (k + 1) * P],
                            rhs=wv[d][:, n * 512 : (n + 1) * 512],
                            start=(d == 0),
                            stop=(d == ND - 1),
                        )
            for j in range(4):
                k = kb * 4 + j
                if j % 2 == 0:
                    nc.vector.tensor_copy(vo[k][:], pss[j][:])
                else:
                    nc.scalar.copy(vo[k][:], pss[j][:])
                qd = nc.sync if j % 2 == 0 else nc.scalar
                qd.dma_start(vb_in[k * P : (k + 1) * P, :], vo[k][:])

        # ---- in-pair AllGather of V (rank order == natural key order) ----
        nc.gpsimd.collective_compute(
            "AllGather",
            mybir.AluOpType.bypass,
            replica_groups=PAIRS,
            ins=[vb_in.ap().opt()],
            outs=[vb_out.ap().opt()],
        )
        vt = [vt_pool.tile([P, D], BF16, tag="vt", name=f"vt{i}") for i in range(NBF)]
        v8 = [
            v8_pool.tile([P, 2 * D], FP8, tag="v8", name=f"v8_{i}") for i in range(3)
        ]
        for k in range(NBF):
            nc.gpsimd.dma_start(vt[k][:], vb_out[k * P : (k + 1) * P, :])
        for k in range(NBF, NS):
            vs = vs_pool.tile([P, D], BF16, tag="vs", name=f"vs{k}")
            nc.gpsimd.dma_start(vs[:], vb_out[k * P : (k + 1) * P, :])
            c2, pl = divmod(k - NBF, 2)
            nc.vector.tensor_copy(v8[c2][:, pl * D : (pl + 1) * D], vs[:])

        # ---- TT proj: tt8[e2][:, j*Q:(j+1)*Q] = (M2^T xq + w2) fp8 ----
        tt8 = [
            tt_pool.tile([P, 2 * Q], FP8, tag="tt", name=f"tt8_{i}")
            for i in range(ND // 2)
        ]
        for e in range(ND):
            pt = psum.tile([P, Q], F32, tag="ps", name=f"tps{e}")
            for d in range(ND):
                for n in range(2):
                    nc.tensor.matmul(
                        pt[:, n * 512 : (n + 1) * 512],
                        lhsT=m2[d][:, e * P : (e + 1) * P],
                        rhs=xq[d][:, n * 512 : (n + 1) * 512],
                        start=(d == 0),
                        stop=(d == ND - 1),
                    )
            dst = tt8[e // 2][:, (e % 2) * Q : (e % 2 + 1) * Q]
            if e % 2 == 0:
                nc.scalar.activation(dst, pt[:], AF.Identity, bias=w2_sb[:, e : e + 1])
            else:
                nc.vector.tensor_scalar_add(dst, pt[:], w2_sb[:, e : e + 1])

        tt3 = [t.rearrange("p (j q) -> p j q", j=2) for t in tt8]
        x83 = [t.rearrange("p (j s) -> p j s", j=2) for t in x8]

        # ---- attention, software-pipelined over 8 q-chunks ----
        def scores_phase(qc):
            """fp8 DoubleRow scores + exp(+mask) + row sums for q-chunk qc."""
            exp_sb = exp_pool.tile([P, S], BF16, tag="exp", name=f"exp{qc}")
            sums = stat_pool.tile([P, 2], F32, tag="sums", name=f"sums{qc}")
            for hf in range(2):
                ps = psum.tile([P, Q], F32, tag="ps", name=f"sps{qc}_{hf}")
                for e2 in range(ND // 2):
                    for n in range(2):
                        off = hf * 1024 + n * 512
                        nc.tensor.matmul(
                            ps[:, n * 512 : (n + 1) * 512],
                            lhsT=tt3[e2][:, :, qc * P : (qc + 1) * P],
                            rhs=x83[e2][:, :, off : off + 512],
                            start=(e2 == 0),
                            stop=(e2 == ND // 2 - 1),
                            perf_mode=DR,
                        )
                if use_mask:
                    mt = m_pool.tile([P, Q], F32, tag="m", name=f"mt{qc}_{hf}")
                    nc.sync.dma_start(
                        mt[:], mask_d[qc * P : (qc + 1) * P, hf * 1024 : (hf + 1) * 1024]
                    )
                    nc.vector.tensor_add(ps[:], ps[:], mt[:])
                nc.scalar.activation(
                    exp_sb[:, hf * 1024 : (hf + 1) * 1024],
                    ps[:],
                    AF.Exp,
                    scale=SCALE,
                    accum_out=sums[:, hf : hf + 1],
                )
            return exp_sb, sums

        def pv_phase(qc, exp_sb, sums):
            """transpose + PV + normalized eviction for q-chunk qc."""
            rsum = stat_pool.tile([P, 1], F32, tag="rsum", name=f"rsum{qc}")
            nc.vector.tensor_add(rsum[:], sums[:, 0:1], sums[:, 1:2])
            rinv = stat_pool.tile([P, 1], F32, tag="rinv", name=f"rinv{qc}")
            nc.vector.reciprocal(rinv[:], rsum[:])
            at_sb = at_pool.tile([P, S], BF16, tag="at", name=f"at{qc}")
            # one xbar transpose for all 16 chunks: out[p, c, q] = exp[q, c*128+p].
            # Issued from the sync queue so it never sits behind the exp
            # ACTIVATEs of later chunks on the in-order Scalar queue.
            nc.sync.dma_start(
                out=at_sb.rearrange("p (c q) -> p c q", q=P),
                in_=exp_sb[:, :],
                transpose=True,
            )
            # fp8 copy of the last 6 transposed-attn chunks; the cast runs on
            # the DVE while the PE does the 10 bf16 chunks, so no latency
            at8 = at8_pool.tile([P, (NS - NBF) * P], FP8, tag="at8", name=f"at8_{qc}")
            nc.vector.tensor_copy(at8[:], at_sb[:, NBF * P : S])
            at83 = at8.rearrange("p (c q) -> p c q", q=P)
            pv = psum.tile([P, D], F32, tag="ps", name=f"pv{qc}")
            for k in range(NBF):
                for n in range(2):
                    nc.tensor.matmul(
                        pv[:, n * 512 : (n + 1) * 512],
                        lhsT=at_sb[:, k * P : (k + 1) * P],
                        rhs=vt[k][:, n * 512 : (n + 1) * 512],
                        start=(k == 0),
                        stop=False,
                    )
            for c2 in range(3):
                v83 = v8[c2].rearrange("p (j e) -> p j e", j=2)
                for n in range(2):
                    nc.tensor.matmul(
                        pv[:, n * 512 : (n + 1) * 512],
                        lhsT=at83[:, 2 * c2 : 2 * c2 + 2, :],
                        rhs=v83[:, :, n * 512 : (n + 1) * 512],
                        start=False,
                        stop=(c2 == 2),
                        perf_mode=DR,
                    )
            # split the normalized eviction + store across two engines/queues
            ot = o_pool.tile([P, D], F32, tag="o", name=f"ot{qc}")
            nc.vector.tensor_scalar_mul(ot[:, 0:512], pv[:, 0:512], rinv[:])
            nc.scalar.mul(ot[:, 512:1024], pv[:, 512:1024], rinv[:])
            nc.sync.dma_start(out_d[qc * P : (qc + 1) * P, 0:512], ot[:, 0:512])
            nc.scalar.dma_start(out_d[qc * P : (qc + 1) * P, 512:1024], ot[:, 512:1024])

        # software pipeline depth 3: pv(0) starts three scores phases in,
        # giving the V AllGather + scatter maximum slack to land
        from collections import deque

        pend = deque([scores_phase(0), scores_phase(1), scores_phase(2)])
        for qc in range(NQ):
            if qc + 3 < NQ:
                pend.append(scores_phase(qc + 3))
            pv_phase(qc, *pend.popleft())

    nc.compile()
    return nc


def _get_nc(use_mask: bool) -> bacc.Bacc:
    if use_mask not in _NC_CACHE:
        _NC_CACHE[use_mask] = _build(use_mask)
    return _NC_CACHE[use_mask]


def kernel(x, mask, Wq, bq, Wk, bk, Wv, bv):
    x = np.asarray(x, dtype=np.float32)
    mask = np.asarray(mask, dtype=np.float32)
    Wq = np.asarray(Wq, dtype=np.float32)
    bq = np.asarray(bq, dtype=np.float32)
    Wk = np.asarray(Wk, dtype=np.float32)
    bk = np.asarray(bk, dtype=np.float32)
    Wv = np.asarray(Wv, dtype=np.float32)
    bv = np.asarray(bv, dtype=np.float32)

    B = x.shape[0]
    use_mask = bool(np.any(mask))

    bf = ml_dtypes.bfloat16
    f8 = ml_dtypes.float8_e4m3
    # scores(i,j) = q_i.k_j - alpha_i with M2 = Wq Wk^T, w2 = Wk bq;
    # alpha_i is a per-row constant that softmax drops.
    m2 = (Wq.astype(np.float64) @ Wk.astype(np.float64).T).astype(bf)
    w2 = (Wk.astype(np.float64) @ bq.astype(np.float64)).astype(np.float32)
    w22 = np.ascontiguousarray(w2.reshape(ND, P).T)
    wv_b = Wv.astype(bf)

    nc = _get_nc(use_mask)

    in_maps = []
    for c in range(8):
        b, h = divmod(c, 2)
        xt = np.ascontiguousarray(x[b].T)  # [D, S], natural key order
        im = {
            "xq": np.ascontiguousarray(xt[:, h * Q : (h + 1) * Q]).astype(bf),
            "x8": xt.astype(f8),
            "m2": m2,
            "wv": wv_b,
            "w22": w22,
        }
        if use_mask:
            im["maskp"] = np.ascontiguousarray(
                mask[h * Q : (h + 1) * Q] / np.float32(SCALE)
            ).astype(np.float32)
        in_maps.append(im)

    res = run_bass_kernel_spmd(nc, in_maps, core_ids=list(range(8)))

    out = np.empty((B, S, D), dtype=np.float32)
    for c in range(8):
        b, h = divmod(c, 2)
        out[b, h * Q : (h + 1) * Q, :] = res.results[c]["out"]
    # bv folded out of the kernel (softmax rows sum to 1): add it back here
    out += bv.reshape(1, 1, D)
    return out


# revision 49
# speedup vs baseline: 1.0819x; 1.0819x over previous
"""Distributed single-head attention for Trainium2 (8 NeuronCores).

Problem: B=4, S=2048, D=1024 fp32 attention:
    q = x@Wq+bq; k = x@Wk+bk; v = x@Wv+bv
    out = softmax(q k^T / sqrt(D) + mask) v

Sharding: core c handles batch c//2, query rows [1024*(c%2), +1024).
The in-pair work split (pair = the two cores of one batch):
  - scores/softmax/PV: each core does its own 1024 query rows over ALL
    2048 keys (x for the whole batch is replicated to the pair as fp8,
    plus a bf16 copy of just the core's own columns, `xq`).
  - V projection: each core projects only its OWN 1024 keys' V and the
    halves are exchanged with ONE in-pair AllGather, overlapped under
    the TT projection + first scores phases. A zero-byte-ish dummy
    AllGather is issued at t=0 so the one-time global CC barrier
    (~20us) burns during the V projection instead of on the critical
    path.
Using a separate `xq` input makes the program identical for both pair
ranks with NATURAL key order everywhere (the AllGather's rank order ==
score column order), so one compiled program serves all 8 cores.

Algebraic tricks:
  - K projection eliminated: scores (up to a per-row constant softmax
    drops) = TT @ x^T with host-precomputed M2 = Wq@Wk^T, w2 = Wk@bq,
    TT = x_q @ M2 + w2.
  - V bias folded out: softmax rows sum to 1 => attn@(xWv+bv)/rsum ==
    attn@(xWv)/rsum + bv; the host adds bv to the final output exactly.

Precision (HW == numpy sim to 3 decimals; gate 2e-2):
  - scores matmul fp8e4m3 DoubleRow (1.0e-2 alone)
  - PV: key chunks 10..15 fp8 DoubleRow, 0..9 bf16 -> 1.674e-2 total
  - everything else bf16 (fp8 on the projections blows the gate)

Per-core PE stream (~592 matmuls at ~216ns):
  V own:   vo[k][128,1024] = xq^T Wv, k=0..7      (128 MM)
  TT:      tt8 fp8 = M2^T xq + w2                 (128 MM)
  scores:  fp8 DoubleRow per q-chunk              (128 MM)
  PV:      10 bf16 + 3 fp8-DR pairs per q-chunk   (208 MM)
"""

from contextlib import ExitStack

import numpy as np
import ml_dtypes

import concourse.bass as bass
import concourse.tile as tile
import concourse.mybir as mybir
from concourse import bacc
from concourse.bass_utils import run_bass_kernel_spmd

BF16 = mybir.dt.bfloat16
FP8 = mybir.dt.float8e4
F32 = mybir.dt.float32
AF = mybir.ActivationFunctionType
DR = mybir.MatmulPerfMode.DoubleRow

D = 1024  # model dim (= contraction dim)
S = 2048  # full sequence (keys)
Q = 1024  # queries per core
P = 128  # partitions
ND = D // P  # 8 d-chunks
NS = S // P  # 16 key chunks
NQ = Q // P  # 8 query chunks
NBF = 10  # key chunks kept bf16 in PV; chunks NBF..15 go fp8 DoubleRow
SCALE = 1.0 / float(np.sqrt(np.float32(D)))
PAIRS = [[0, 1], [2, 3], [4, 5], [6, 7]]

_NC_CACHE: dict[bool, bacc.Bacc] = {}


def _build(use_mask: bool) -> bacc.Bacc:
    nc = bacc.Bacc("TRN2", target_bir_lowering=False, debug=False, num_devices=8)

    xq_d = nc.dram_tensor("xq", [D, Q], BF16, kind="ExternalInput")
    x8_d = nc.dram_tensor("x8", [D, S], FP8, kind="ExternalInput")
    m2_d = nc.dram_tensor("m2", [D, D], BF16, kind="ExternalInput")
    wv_d = nc.dram_tensor("wv", [D, D], BF16, kind="ExternalInput")
    w2_d = nc.dram_tensor("w22", [P, ND], F32, kind="ExternalInput")
    if use_mask:
        mask_d = nc.dram_tensor("maskp", [Q, S], F32, kind="ExternalInput")
    out_d = nc.dram_tensor("out", [Q, D], F32, kind="ExternalOutput")

    # collective bounce buffers + a tiny dummy to pre-fire the CC barrier;
    # V is exchanged in two pipelined halves of 4 own-chunks each
    dmy_in = nc.dram_tensor("dmy_in", [1, 64], BF16)
    dmy_out = nc.dram_tensor("dmy_out", [2, 64], BF16)
    vb_in = [nc.dram_tensor(f"vb_in{i}", [Q // 2, D], BF16) for i in range(2)]
    vb_out = [nc.dram_tensor(f"vb_out{i}", [Q, D], BF16) for i in range(2)]

    with tile.TileContext(nc) as tc, ExitStack() as ctx:
        xq_pool = ctx.enter_context(tc.tile_pool(name="xq", bufs=ND))
        x8_pool = ctx.enter_context(tc.tile_pool(name="x8", bufs=ND // 2))
        m2_pool = ctx.enter_context(tc.tile_pool(name="m2", bufs=ND))
        wv_pool = ctx.enter_context(tc.tile_pool(name="wv", bufs=ND))
        tt_pool = ctx.enter_context(tc.tile_pool(name="tt", bufs=ND // 2))
        vo_pool = ctx.enter_context(tc.tile_pool(name="vo", bufs=ND))
        vt_pool = ctx.enter_context(tc.tile_pool(name="vt", bufs=NBF))
        vs_pool = ctx.enter_context(tc.tile_pool(name="vs", bufs=3))
        v8_pool = ctx.enter_context(tc.tile_pool(name="v8", bufs=3))
        at8_pool = ctx.enter_context(tc.tile_pool(name="at8", bufs=2))
        const_pool = ctx.enter_context(tc.tile_pool(name="const", bufs=1))
        exp_pool = ctx.enter_context(tc.tile_pool(name="exp", bufs=4))
        at_pool = ctx.enter_context(tc.tile_pool(name="at", bufs=2))
        stat_pool = ctx.enter_context(tc.tile_pool(name="stat", bufs=12))
        o_pool = ctx.enter_context(tc.tile_pool(name="o", bufs=2))
        if use_mask:
            m_pool = ctx.enter_context(tc.tile_pool(name="m", bufs=2))
        psum = ctx.enter_context(tc.tile_pool(name="psum", bufs=4, space="PSUM"))

        # ---- input loads ----
        # gpsimd: xq halves (V+TT critical path), w22, then x8 (first read
        #         only at the scores phase ~70us in)
        # sync/scalar: wv halves split over both queues, then m2 on sync
        xq = [xq_pool.tile([P, Q], BF16, tag="xq", name=f"xq{i}") for i in range(ND)]
        wv = [wv_pool.tile([P, D], BF16, tag="wv", name=f"wv{i}") for i in range(ND)]
        m2 = [m2_pool.tile([P, D], BF16, tag="m2", name=f"m2{i}") for i in range(ND)]
        for h in range(2):
            for d in range(ND):
                nc.gpsimd.dma_start(
                    xq[d][:, h * 512 : (h + 1) * 512],
                    xq_d[d * P : (d + 1) * P, h * 512 : (h + 1) * 512],
                )
        for d in range(ND):
            q = nc.sync if d % 2 == 0 else nc.scalar
            for n in range(2):
                q.dma_start(
                    wv[d][:, n * 512 : (n + 1) * 512],
                    wv_d[d * P : (d + 1) * P, n * 512 : (n + 1) * 512],
                )
        for d in range(ND):
            nc.sync.dma_start(m2[d][:], m2_d[d * P : (d + 1) * P, :])

        x8 = [
            x8_pool.tile([P, 2 * S], FP8, tag="x8", name=f"x8_{i}")
            for i in range(ND // 2)
        ]
        w2_sb = const_pool.tile([P, ND], F32, tag="w2")
        nc.gpsimd.dma_start(w2_sb[:], w2_d[:, :])
        for e2 in range(ND // 2):
            for j in range(2):
                nc.gpsimd.dma_start(
                    x8[e2][:, j * S : (j + 1) * S],
                    x8_d[(2 * e2 + j) * P : (2 * e2 + j + 1) * P, :],
                )

        # ---- V own-half proj + two pipelined in-pair AllGathers ----
        # Half h carries own chunks 4h..4h+3; AllGather rank order makes
        # out half 0 = natural chunks {0..3, 8..11}, half 1 = {4..7, 12..15}.
        vo = [vo_pool.tile([P, D], BF16, tag="vo", name=f"vo{i}") for i in range(ND)]
        vt = [vt_pool.tile([P, D], BF16, tag="vt", name=f"vt{i}") for i in range(NBF)]
        v8 = [
            v8_pool.tile([P, 2 * D], FP8, tag="v8", name=f"v8_{i}") for i in range(3)
        ]

        def scatter_chunk(src_ap, k):
            """route gathered natural chunk k to its PV home (bf16 or fp8)."""
            if k < NBF:
                nc.gpsimd.dma_start(vt[k][:], src_ap)
            else:
                vs = vs_pool.tile([P, D], BF16, tag="vs", name=f"vs{k}")
                nc.gpsimd.dma_start(vs[:], src_ap)
                c2, pl = divmod(k - NBF, 2)
                nc.vector.tensor_copy(v8[c2][:, pl * D : (pl + 1) * D], vs[:])

        for kb in range(2):
            pss = [
                psum.tile([P, D], F32, tag="ps", name=f"vps{kb}_{j}")
                for j in range(4)
            ]
            for d in range(ND):
                for j in range(4):
                    k = kb * 4 + j
                    for n in range(2):
                        nc.tensor.matmul(
                            pss[j][:, n * 512 : (n + 1) * 512],
                            lhsT=xq[d][:, k * P : (k + 1) * P],
                            rhs=wv[d][:, n * 512 : (n + 1) * 512],
                            start=(d == 0),
                            stop=(d == ND - 1),
                        )
            for j in range(4):
                k = kb * 4 + j
                if j % 2 == 0:
                    nc.vector.tensor_copy(vo[k][:], pss[j][:])
                else:
                    nc.scalar.copy(vo[k][:], pss[j][:])
                qd = nc.sync if j % 2 == 0 else nc.scalar
                qd.dma_start(vb_in[kb][j * P : (j + 1) * P, :], vo[k][:])
        # both gather triggers are emitted BEFORE any scatter: the scatters
        # block the gpsimd queue until their gather's data lands, and a
        # trigger queued behind them would inherit that delay
        for kb in range(2):
            nc.gpsimd.collective_compute(
                "AllGather",
                mybir.AluOpType.bypass,
                replica_groups=PAIRS,
                ins=[vb_in[kb].ap().opt()],
                outs=[vb_out[kb].ap().opt()],
            )
        for kb in range(2):
            for j in range(4):
                scatter_chunk(vb_out[kb][j * P : (j + 1) * P, :], kb * 4 + j)
            for j in range(4):
                scatter_chunk(
                    vb_out[kb][Q // 2 + j * P : Q // 2 + (j + 1) * P, :],
                    8 + kb * 4 + j,
                )

        # ---- TT proj: tt8[e2][:, j*Q:(j+1)*Q] = (M2^T xq + w2) fp8 ----
        tt8 = [
            tt_pool.tile([P, 2 * Q], FP8, tag="tt", name=f"tt8_{i}")
            for i in range(ND // 2)
        ]
        for e in range(ND):
            pt = psum.tile([P, Q], F32, tag="ps", name=f"tps{e}")
            for d in range(ND):
                for n in range(2):
                    nc.tensor.matmul(
                        pt[:, n * 512 : (n + 1) * 512],
                        lhsT=m2[d][:, e * P : (e + 1) * P],
                        rhs=xq[d][:, n * 512 : (n + 1) * 512],
                        start=(d == 0),
                        stop=(d == ND - 1),
                    )
            dst = tt8[e // 2][:, (e % 2) * Q : (e % 2 + 1) * Q]
            if e % 2 == 0:
                nc.scalar.activation(dst, pt[:], AF.Identity, bias=w2_sb[:, e : e + 1])
            else:
                nc.vector.tensor_scalar_add(dst, pt[:], w2_sb[:, e : e + 1])

        tt3 = [t.rearrange("p (j q) -> p j q", j=2) for t in tt8]
        x83 = [t.rearrange("p (j s) -> p j s", j=2) for t in x8]

        # ---- attention, software-pipelined over 8 q-chunks ----
        def scores_phase(qc):
            """fp8 DoubleRow scores + exp(+mask) + row sums for q-chunk qc."""
            exp_sb = exp_pool.tile([P, S], BF16, tag="exp", name=f"exp{qc}")
            sums = stat_pool.tile([P, 2], F32, tag="sums", name=f"sums{qc}")
            for hf in range(2):
                ps = psum.tile([P, Q], F32, tag="ps", name=f"sps{qc}_{hf}")
                for e2 in range(ND // 2):
                    for n in range(2):
                        off = hf * 1024 + n * 512
                        nc.tensor.matmul(
                            ps[:, n * 512 : (n + 1) * 512],
                            lhsT=tt3[e2][:, :, qc * P : (qc + 1) * P],
                            rhs=x83[e2][:, :, off : off + 512],
                            start=(e2 == 0),
                            stop=(e2 == ND // 2 - 1),
                            perf_mode=DR,
                        )
                if use_mask:
                    mt = m_pool.tile([P, Q], F32, tag="m", name=f"mt{qc}_{hf}")
                    nc.sync.dma_start(
                        mt[:], mask_d[qc * P : (qc + 1) * P, hf * 1024 : (hf + 1) * 1024]
                    )
                    nc.vector.tensor_add(ps[:], ps[:], mt[:])
                nc.scalar.activation(
                    exp_sb[:, hf * 1024 : (hf + 1) * 1024],
                    ps[:],
                    AF.Exp,
                    scale=SCALE,
                    accum_out=sums[:, hf : hf + 1],
                )
            return exp_sb, sums

        def pv_phase(qc, exp_sb, sums):
            """transpose + PV + normalized eviction for q-chunk qc."""
            rsum = stat_pool.tile([P, 1], F32, tag="rsum", name=f"rsum{qc}")
            nc.vector.tensor_add(rsum[:], sums[:, 0:1], sums[:, 1:2])
            rinv = stat_pool.tile([P, 1], F32, tag="rinv", name=f"rinv{qc}")
            nc.vector.reciprocal(rinv[:], rsum[:])
            at_sb = at_pool.tile([P, S], BF16, tag="at", name=f"at{qc}")
            # one xbar transpose for all 16 chunks: out[p, c, q] = exp[q, c*128+p].
            # Issued from the sync queue so it never sits behind the exp
            # ACTIVATEs of later chunks on the in-order Scalar queue.
            nc.sync.dma_start(
                out=at_sb.rearrange("p (c q) -> p c q", q=P),
                in_=exp_sb[:, :],
                transpose=True,
            )
            # fp8 copy of the last 6 transposed-attn chunks; the cast runs on
            # the DVE while the PE does the 10 bf16 chunks, so no latency
            at8 = at8_pool.tile([P, (NS - NBF) * P], FP8, tag="at8", name=f"at8_{qc}")
            nc.vector.tensor_copy(at8[:], at_sb[:, NBF * P : S])
            at83 = at8.rearrange("p (c q) -> p c q", q=P)
            pv = psum.tile([P, D], F32, tag="ps", name=f"pv{qc}")
            # gather-0 chunks {0-3, 8, 9} first: gather-1 lands ~15us later
            pv_order = [0, 1, 2, 3, 8, 9, 4, 5, 6, 7]
            for i, k in enumerate(pv_order):
                for n in range(2):
                    nc.tensor.matmul(
                        pv[:, n * 512 : (n + 1) * 512],
                        lhsT=at_sb[:, k * P : (k + 1) * P],
                        rhs=vt[k][:, n * 512 : (n + 1) * 512],
                        start=(i == 0),
                        stop=False,
                    )
            for c2 in range(3):
                v83 = v8[c2].rearrange("p (j e) -> p j e", j=2)
                for n in range(2):
                    nc.tensor.matmul(
                        pv[:, n * 512 : (n + 1) * 512],
                        lhsT=at83[:, 2 * c2 : 2 * c2 + 2, :],
                        rhs=v83[:, :, n * 512 : (n + 1) * 512],
                        start=False,
                        stop=(c2 == 2),
                        perf_mode=DR,
                    )
            # split the normalized eviction + store across two engines/queues
            ot = o_pool.tile([P, D], F32, tag="o", name=f"ot{qc}")
            nc.vector.tensor_scalar_mul(ot[:, 0:512], pv[:, 0:512], rinv[:])
            nc.scalar.mul(ot[:, 512:1024], pv[:, 512:1024], rinv[:])
            nc.sync.dma_start(out_d[qc * P : (qc + 1) * P, 0:512], ot[:, 0:512])
            nc.scalar.dma_start(out_d[qc * P : (qc + 1) * P, 512:1024], ot[:, 512:1024])

        # software pipeline depth 3: pv(0) starts three scores phases in,
        # giving the V AllGather + scatter maximum slack to land
        from collections import deque

        pend = deque([scores_phase(0), scores_phase(1), scores_phase(2)])
        for qc in range(NQ):
            if qc + 3 < NQ:
                pend.append(scores_phase(qc + 3))
            pv_phase(qc, *pend.popleft())

    nc.compile()
    return nc


def _get_nc(use_mask: bool) -> bacc.Bacc:
    if use_mask not in _NC_CACHE:
        _NC_CACHE[use_mask] = _build(use_mask)
    return _NC_CACHE[use_mask]


def kernel(x, mask, Wq, bq, Wk, bk, Wv, bv):
    x = np.asarray(x, dtype=np.float32)
    mask = np.asarray(mask, dtype=np.float32)
    Wq = np.asarray(Wq, dtype=np.float32)
    bq = np.asarray(bq, dtype=np.float32)
    Wk = np.asarray(Wk, dtype=np.float32)
    bk = np.asarray(bk, dtype=np.float32)
    Wv = np.asarray(Wv, dtype=np.float32)
    bv = np.asarray(bv, dtype=np.float32)

    B = x.shape[0]
    use_mask = bool(np.any(mask))

    bf = ml_dtypes.bfloat16
    f8 = ml_dtypes.float8_e4m3
    # scores(i,j) = q_i.k_j - alpha_i with M2 = Wq Wk^T, w2 = Wk bq;
    # alpha_i is a per-row constant that softmax drops.
    m2 = (Wq.astype(np.float64) @ Wk.astype(np.float64).T).astype(bf)
    w2 = (Wk.astype(np.float64) @ bq.astype(np.float64)).astype(np.float32)
    w22 = np.ascontiguousarray(w2.reshape(ND, P).T)
    wv_b = Wv.astype(bf)

    nc = _get_nc(use_mask)

    in_maps = []
    for c in range(8):
        b, h = divmod(c, 2)
        xt = np.ascontiguousarray(x[b].T)  # [D, S], natural key order
        im = {
            "xq": np.ascontiguousarray(xt[:, h * Q : (h + 1) * Q]).astype(bf),
            "x8": xt.astype(f8),
            "m2": m2,
            "wv": wv_b,
            "w22": w22,
        }
        if use_mask:
            im["maskp"] = np.ascontiguousarray(
                mask[h * Q : (h + 1) * Q] / np.float32(SCALE)
            ).astype(np.float32)
        in_maps.append(im)

    res = run_bass_kernel_spmd(nc, in_maps, core_ids=list(range(8)))

    out = np.empty((B, S, D), dtype=np.float32)
    for c in range(8):
        b, h = divmod(c, 2)
        out[b, h * Q : (h + 1) * Q, :] = res.results[c]["out"]
    # bv folded out of the kernel (softmax rows sum to 1): add it back here
    out += bv.reshape(1, 1, D)
    return out
